# revision 1
# baseline (speedup 1.0000x reference)
"""EntropyLinear Trainium2 kernel (8-core SPMD, batch-sharded).

Computes, for x[B,IN], weight[C,OUT,IN], bias[C,1,OUT]:
    gamma[c,i]      = sum_o |W[c,o,i]|
    alpha_norm[c,i] = exp((gamma[c,i] - max_i gamma[c,i]) / T)
                      (softmax / rowmax(softmax) -- denominator cancels)
    y[b,c,o]        = sum_i x[b,i] * alpha_norm[c,i] * W[c,o,i] + bias[c,0,o]

Strategy: data-parallel over batch. Each of the 8 cores gets 1024 rows of x
plus the full weight/bias (tiny), folds alpha_norm into a transposed copy of
W on-chip, and runs the per-class GEMM as one [1024,256] x [256,1600] f32r
matmul.

The whole prologue is pipelined per W piece (4 triples of 128-row groups +
the 64-row tail). Gamma rows for a class depend only on that class's own
128 W rows, so each piece independently runs:
  DMA -> ACT abs -> PE block-mask matmul (partition-reduce over o, into a
  per-class-row PSUM accumulator) -> DVE rowmax -> ACT exp -> PE transpose
  of its alpha rows -> DVE scale-fused W-transpose copies (wm) -> the
  matching output chunk of b-tile 0 -> its half of the first store.
Stores then stream back-to-back; steady-state epilogue alternates DVE
fused psum+bias adds (2 of 5 chunks) and ACT plain copies whose bias is
pre-injected in PSUM by a rank-1 (ones x bias) matmul. A burst of junk
transposes warms the PE clock gate during the load window.
"""

import os
from contextlib import ExitStack

import numpy as np

import concourse.bass as bass
import concourse.tile as tile
from concourse import masks, mybir
from concourse.bass_utils import run_bass_kernel_spmd

# ---------------------------------------------------------------------------
# Workaround for this walrus build's 1-sync-wait-per-instruction limit:
# Tile's scheduler (and its exit drain) may attach several semaphore waits to
# one instruction; walrus codegen rejects that ("Too many sync wait
# commands"). Post-pass: hoist extra waits onto same-engine NOPs emitted
# immediately before the instruction (same engine stream => same semantics).
# ---------------------------------------------------------------------------


def _split_multi_waits(nc):
    for fn in nc.m.functions:
        for blk in fn.blocks:
            out = []
            changed = False
            for inst in list(blk.instructions):
                si = inst.sync_info
                waits = list(si.on_wait) if si is not None and si.on_wait else []
                if len(waits) > 1:
                    changed = True
                    for w in waits[:-1]:
                        nop = mybir.InstNoOp(
                            name=nc.get_next_instruction_name(), ins=[], outs=[])
                        nop.engine = inst.engine
                        nop.sync_info = mybir.SyncInfo(on_wait=[w], on_update=[])
                        nc.register_instruction(nop)
                        out.append(nop)
                    upd = list(si.on_update) if si.on_update else []
                    inst.sync_info = mybir.SyncInfo(
                        on_wait=[waits[-1]], on_update=upd)
                out.append(inst)
            if changed:
                blk.instructions = out

# ---------------------------------------------------------------------------

B, IN, OUT, C = 8192, 256, 32, 50
TEMPERATURE = 0.6
N_CORES = 8
BS = B // N_CORES          # rows of x per core
CO = C * OUT               # 1600 fused (class, out) columns
F32 = mybir.dt.float32
F32R = mybir.dt.float32r
BF16 = mybir.dt.bfloat16

N_BT = BS // 128           # b-tiles per core
N_KH = IN // 128           # contraction chunks
CO_CHUNK = 320             # psum chunk (10 classes, 1280B -> one psum bank)
N_CC = CO // CO_CHUNK
N_GRP = (CO + 127) // 128  # W co-groups (13: 12 full + 1 half)
CPG = 128 // OUT           # classes per full W group (4)
GPP = int(os.environ.get("EL_GPP", "3"))  # W groups per pipeline piece
N_PC = (12 + GPP - 1) // GPP + 1           # full pieces + the 64-row tail

# "f32r" (full-rate fp32 path), "f32" (4x slower, exact), "bf16"
MM_MODE = os.environ.get("EL_MM_MODE", "f32r")
Y_DVE_OF_5 = int(os.environ.get("EL_Y_DVE_OF_5", "2"))   # y chunks on DVE per 5
XCOPY_ACT = os.environ.get("EL_XCOPY_ACT", "0") == "1"   # x copies on ACT
T0_ACT = os.environ.get("EL_T0_ACT", "1") == "1"         # t=0 all-ACT epilogue

_CACHE = {}


def _build(mode, repeat=1):
    nc = bass.Bass(trn_type="TRN2", target_bir_lowering=False, debug=False,
                   num_devices=N_CORES)
    x_d = nc.dram_tensor("x", [BS, IN], F32, kind="ExternalInput").ap()
    w_d = nc.dram_tensor("weight", [C, OUT, IN], F32, kind="ExternalInput").ap()
    b_d = nc.dram_tensor("bias", [C, 1, OUT], F32, kind="ExternalInput").ap()
    y_d = nc.dram_tensor("y", [BS, C, OUT], F32, kind="ExternalOutput").ap()

    w_flat = w_d.rearrange("c o i -> (c o) i")      # [1600, 256]
    y_flat = y_d.rearrange("b c o -> b (c o)")      # [BS, 1600]

    # dtype of matmul operand tiles; f32r rounding happens in the DVE ops
    # that produce them
    mm_dt = {"f32": F32, "f32r": F32R, "bf16": BF16}[mode]

    with tile.TileContext(nc) as tc, ExitStack() as ctx:
      const_p = ctx.enter_context(tc.tile_pool(name="const", bufs=1))
      wn_p = ctx.enter_context(tc.tile_pool(name="wn", bufs=1))
      small_p = ctx.enter_context(tc.tile_pool(name="small", bufs=1))
      x_p = ctx.enter_context(tc.tile_pool(name="x", bufs=1))
      xt_p = ctx.enter_context(tc.tile_pool(name="xt", bufs=16))
      y_p = ctx.enter_context(tc.tile_pool(name="y", bufs=3))
      ps_tp = ctx.enter_context(tc.tile_pool(name="ps_tp", bufs=2, space="PSUM"))
      ps_g = ctx.enter_context(tc.tile_pool(name="ps_g", bufs=2, space="PSUM"))
      ps_y = ctx.enter_context(tc.tile_pool(name="ps_y", bufs=4, space="PSUM"))
      rep_cm = tc.For_i(0, repeat, 1) if repeat > 1 else None
      if rep_cm is not None:
          rep_cm.__enter__()
      if True:
          # ---- constants ----
          ident = const_p.tile([128, 128], F32, tag="ident")
          masks.make_identity(nc, ident[:])
          # Sliding-window mask for the per-class partition reduction.
          # mstore[p, CW + j] = 1 iff p // OUT == j; all else 0. Group g uses
          # the [128, CW] window starting at column CW - CPG*g, which places
          # the identity block exactly at out-classes CPG*g..CPG*g+CPG.
          CW = 32     # per-piece gamma psum partitions (>= GPP*CPG)
          mstore_f = const_p.tile([128, 2 * CW], F32, tag="mstore_f")
          nc.vector.memset(mstore_f[:], 0.0)
          # block-identity derived by summing OUT-wide column blocks of the
          # identity matrix
          nc.vector.tensor_reduce(
              mstore_f[:, CW:CW + CPG],
              ident[:].rearrange("p (j q) -> p j q", q=OUT),
              axis=mybir.AxisListType.X, op=mybir.AluOpType.add)
          mstore = const_p.tile([128, 2 * CW], F32R, tag="mstore")
          nc.vector.tensor_copy(mstore[:], mstore_f[:])
          ones_f = const_p.tile([1, 128], F32, tag="ones_f")
          nc.vector.memset(ones_f[:], 1.0)
          ones_r = const_p.tile([1, 128], F32R, tag="ones_r")
          nc.vector.tensor_copy(ones_r[:], ones_f[:])

          # ---- PE warmup: junk transposes so the HAM clock gate opens ----
          for _ in range(14):
              warm_ps = ps_tp.tile([128, 256], F32, tag="tp", name="tp")
              nc.tensor.transpose(warm_ps[:, :128], ident[:], ident[:])

          # ---- loads: W piece 0, x(t=0,1), W pieces 1-3 + tail, bias,
          #      x rest. Each W triple is one 3D-AP DMA. ----
          wbig = wn_p.tile([128, N_GRP * IN], F32, tag="wbig")
          xbig = x_p.tile([128, N_BT * IN], F32, tag="xbig")

          def load_w_piece(p):
              g0 = p * GPP
              g1 = min(g0 + GPP, 12)
              if g0 < 12:
                  nc.sync.dma_start(
                      wbig[:, g0 * IN:g1 * IN].rearrange(
                          "p (g i) -> p g i", i=IN),
                      w_flat[g0 * 128:g1 * 128, :].rearrange(
                          "(g p) i -> p g i", p=128))
              if g1 * 128 < CO <= (g0 + GPP) * 128:
                  nc.sync.dma_start(wbig[:64, 12 * IN:], w_flat[1536:1600, :])

          load_w_piece(0)
          nc.sync.dma_start(
              xbig[:, :2 * IN].rearrange("p (t i) -> p t i", i=IN),
              x_d[0:256, :].rearrange("(t p) i -> p t i", p=128))
          for p in range(1, N_PC):
              load_w_piece(p)
          bias_row = const_p.tile([1, CO], F32, tag="bias_row")
          nc.sync.dma_start(bias_row[:], b_d.rearrange("c u o -> u (c o)"))
          nc.sync.dma_start(
              xbig[:, 2 * IN:].rearrange("p (t i) -> p t i", i=IN),
              x_d[256:, :].rearrange("(t p) i -> p t i", p=128))

          wn = [wbig[:, g * IN:(g + 1) * IN] for g in range(N_GRP)]
          xn = [xbig[:, t * IN:(t + 1) * IN] for t in range(N_BT)]

          # ---- per-piece pipeline state ----
          awbig = wn_p.tile([128, N_GRP * IN], F32R, tag="awbig")
          ant = [small_p.tile([128, C], F32, tag=f"ant{h}", name=f"ant{h}")
                 for h in range(N_KH)]
          wm = [const_p.tile([128, CO], mm_dt, tag=f"wm{h}", name=f"wm{h}")
                for h in range(N_KH)]
          bias_r = const_p.tile([1, CO], F32R, tag="bias_r")
          nc.vector.tensor_copy(bias_r[:], bias_row[:])
          bias_rep = const_p.tile([128, CO], F32, tag="bias_rep")
          if not T0_ACT:
              for n in range(N_CC):
                  sl = slice(n * CO_CHUNK, (n + 1) * CO_CHUNK)
                  ps = ps_y.tile([128, CO_CHUNK], F32, tag="ps", name="ps")
                  nc.tensor.matmul(ps[:], ones_r[:], bias_r[:, sl],
                                   start=True, stop=True)
                  nc.scalar.copy(bias_rep[:, sl], ps[:])

          def x_transpose(t):
              xtt = []
              for h in range(N_KH):
                  tp = ps_tp.tile([128, 256], F32, tag="tp", name="tp")
                  nc.tensor.transpose(tp[:, :128],
                                      xn[t][:, h * 128:(h + 1) * 128],
                                      ident[:])
                  xte = xt_p.tile([128, 128], mm_dt, tag="xte", name="xte")
                  if XCOPY_ACT:
                      nc.scalar.copy(xte[:], tp[:, :128])
                  else:
                      nc.vector.tensor_copy(xte[:], tp[:, :128])
                  xtt.append(xte)
              return xtt

          def w_piece_pipeline(p):
              """abs -> gamma rows -> alpha rows -> anT cols -> wm cols."""
              g0 = p * GPP
              g1 = min(g0 + GPP, N_GRP)
              col0, col1 = g0 * 128, min(CO, g1 * 128)
              ncls = (col1 - col0) // OUT
              cl0 = g0 * CPG
              pgl = min(128, CO - (g1 - 1) * 128)      # rows in last group
              rows = slice(cl0, cl0 + ncls)
              # |W| for the piece (one wide ACT op; rows beyond pgl unused)
              nc.scalar.activation(awbig[:pgl, g0 * IN:g1 * IN],
                                   wbig[:pgl, g0 * IN:g1 * IN],
                                   mybir.ActivationFunctionType.Abs)
              # per-group partition reduction over o into this piece's
              # local gamma rows (base partition 0)
              gps = ps_g.tile([CW, IN], F32, tag="gps", name="gps")
              for g in range(g0, g1):
                  pg = min(128, CO - g * 128)
                  loc = g - g0
                  nc.tensor.matmul(
                      gps[:, :],
                      mstore[:pg, CW - CPG * loc: CW * 2 - CPG * loc],
                      awbig[:pg, g * IN:(g + 1) * IN],
                      start=(loc == 0), stop=(g == g1 - 1),
                      skip_group_check=True)
              # alpha rows for this piece's classes
              gm = small_p.tile([CW, 1], F32, tag="gm", name="gm", bufs=2)
              nc.vector.tensor_reduce(gm[:ncls], gps[:ncls, :],
                                      axis=mybir.AxisListType.X,
                                      op=mybir.AluOpType.max)
              nb = small_p.tile([CW, 1], F32, tag="nb", name="nb", bufs=2)
              nc.vector.tensor_scalar_mul(nb[:ncls], gm[:ncls],
                                          -1.0 / TEMPERATURE)
              anp = small_p.tile([CW, IN], F32, tag="anp", name="anp",
                                 bufs=2)
              nc.scalar.activation(anp[:ncls, :], gps[:ncls, :],
                                   mybir.ActivationFunctionType.Exp,
                                   bias=nb[:ncls], scale=1.0 / TEMPERATURE)
              # transpose alpha rows into anT columns, then scale-fused
              # W-transpose into wm
              for h in range(N_KH):
                  tp = ps_tp.tile([128, 256], F32, tag="tp", name="tp")
                  nc.tensor.transpose(tp[:, :ncls],
                                      anp[:ncls, h * 128:(h + 1) * 128],
                                      ident[:ncls, :ncls])
                  nc.vector.tensor_copy(ant[h][:, rows], tp[:, :ncls])
                  for ga in range(g0, g1, 2):
                      gb = min(ga + 2, g1)
                      bcol0, bcol1 = ga * 128, min(CO, gb * 128)
                      bncls = (bcol1 - bcol0) // OUT
                      bc0 = ga * CPG
                      tpw = ps_tp.tile([128, 256], F32, tag="tp", name="tp")
                      for g in range(ga, gb):
                          pg = min(128, CO - g * 128)
                          off = (g - ga) * 128
                          nc.tensor.transpose(
                              tpw[:, off:off + pg],
                              wn[g][:pg, h * 128:(h + 1) * 128],
                              ident[:pg, :pg])
                      nc.vector.tensor_tensor(
                          wm[h][:, bcol0:bcol1].rearrange(
                              "p (c o) -> p c o", o=OUT),
                          tpw[:, :bcol1 - bcol0].rearrange(
                              "p (c o) -> p c o", o=OUT),
                          ant[h][:, bc0:bc0 + bncls].unsqueeze(2).broadcast_to(
                              [128, bncls, OUT]),
                          op=mybir.AluOpType.mult)

          def y_chunk(t, n, y_sb, on_act):
              sl = slice(n * CO_CHUNK, (n + 1) * CO_CHUNK)
              ps = ps_y.tile([128, CO_CHUNK], F32, tag="ps", name="ps")
              if on_act:
                  nc.tensor.matmul(ps[:], ones_r[:], bias_r[:, sl],
                                   start=True, stop=False)
              for h in range(N_KH):
                  nc.tensor.matmul(ps[:], xt[t][h][:], wm[h][:, sl],
                                   start=(h == 0 and not on_act),
                                   stop=(h == N_KH - 1))
              if on_act:
                  nc.scalar.copy(y_sb[:, sl], ps[:])
              else:
                  nc.vector.tensor_tensor(y_sb[:, sl], ps[:],
                                          bias_rep[:, sl],
                                          op=mybir.AluOpType.add)

          # ---- pipelined prologue: piece p unlocks t=0's chunk p ----
          # chunk n spans co columns [320n, 320n+320) = groups 2.5n..2.5n+2.5,
          # covered by pieces 0..n (piece p covers groups 3p..3p+3)
          xt = []
          y0 = y_p.tile([128, CO], F32, tag="y_sb", name="y_sb")
          done_chunks = 0
          stored_cols = 0
          for p in range(N_PC):
              w_piece_pipeline(p)
              if p == 0:
                  xt.extend(x_transpose(t) for t in range(2))
              cov = min(CO, (p + 1) * GPP * 128)     # wm columns ready
              if p == N_PC - 1:
                  cov = CO
              while done_chunks < N_CC and (done_chunks + 1) * CO_CHUNK <= cov:
                  y_chunk(0, done_chunks, y0, T0_ACT)
                  done_chunks += 1
              if done_chunks >= 2 and stored_cols == 0:
                  stored_cols = done_chunks * CO_CHUNK
                  nc.sync.dma_start(y_flat[0:128, :stored_cols],
                                    y0[:, :stored_cols])
          nc.sync.dma_start(y_flat[0:128, stored_cols:], y0[:, stored_cols:])

          # remaining x transposes
          xt += [x_transpose(t) for t in range(2, N_BT)]

          # bias_rep via rank-1 matmuls (only DVE chunks need it; first use
          # is t=1 -- build it behind t=0's epilogue)
          if T0_ACT and Y_DVE_OF_5 > 0:
              for n in range(N_CC):
                  sl = slice(n * CO_CHUNK, (n + 1) * CO_CHUNK)
                  ps = ps_y.tile([128, CO_CHUNK], F32, tag="ps", name="ps")
                  nc.tensor.matmul(ps[:], ones_r[:], bias_r[:, sl],
                                   start=True, stop=True)
                  nc.scalar.copy(bias_rep[:, sl], ps[:])

          for t in range(1, N_BT):
              y_sb = y_p.tile([128, CO], F32, tag="y_sb", name="y_sb")
              for n in range(N_CC):
                  on_act = (t * N_CC + n) % 5 >= Y_DVE_OF_5
                  y_chunk(t, n, y_sb, on_act)
              nc.sync.dma_start(y_flat[t * 128:(t + 1) * 128, :], y_sb[:])

      if rep_cm is not None:
          rep_cm.__exit__(None, None, None)

    _split_multi_waits(nc)
    return nc


def _get_nc(mode, repeat=1):
    key = (mode, repeat)
    if key not in _CACHE:
        _CACHE[key] = _build(mode, repeat)
    return _CACHE[key]


def kernel(x: np.ndarray, weight: np.ndarray, bias: np.ndarray,
           _trace: bool = False, _repeat: int = 1):
    nc = _get_nc(MM_MODE, _repeat)
    x = np.ascontiguousarray(x, dtype=np.float32)
    weight = np.ascontiguousarray(weight, dtype=np.float32)
    bias = np.ascontiguousarray(bias, dtype=np.float32)
    in_maps = [
        {"x": x[i * BS:(i + 1) * BS], "weight": weight, "bias": bias}
        for i in range(N_CORES)
    ]
    res = run_bass_kernel_spmd(nc, in_maps, list(range(N_CORES)), trace=_trace)
    out = np.concatenate([res.results[i]["y"] for i in range(N_CORES)], axis=0)
    if _trace:
        return out, res
    return out

